# revision 16
# baseline (speedup 1.0000x reference)
"""Trainium2 Bass kernel for AdjacencyMatrixLearning.

Computes, per (b, t):
    f1 = relu(freq @ W1.T + b1); f2 = relu(freq @ W2.T + b2)   # freq: [C, 64]
    t1 = relu(time @ W3.T + b3); t2 = relu(time @ W4.T + b4)   # time: [C, 128]
    A  = (f1 @ f2.T + t1 @ t2.T) / sqrt(C)
    out = softmax(A, axis=-1)                                   # [C, C]

Sharding: pure data-parallel over B across 8 NeuronCores.

Device-side strategy (per core, B_loc=512, T=9, C=22 -> 4608 positions):
  - Inputs pre-transposed on host to x[d, (b, t, c)], bf16.
  - freq is "folded" to K=128: column j of a chunk pairs with column j+176,
    stacked along partitions; weights are zero-padded block-diagonal so every
    stage-1 matmul is tile_size (128, 64). The f-matmuls run on column-tile
    (0,0) (psum partitions 0-63), the t-matmuls on (0,64), concurrently,
    producing g1 = [f1; t1], g2 = [f2; t2] directly.
  - Stage 2: per-position K=128 matmul A = g1_blk.T @ g2_blk -> PSUM [22, 22]
    at partition base 32*(q%4) so softmax access patterns stay affine.
    A-psum units span 4 chunks to amortize softmax instruction overheads.
  - Softmax without max-subtraction (logits bounded): exp on ScalarE with the
    1/sqrt(C) scale folded in, sum+recip on VectorE, multiply on GpSimd.
  - Output is written to DRAM in an SBUF-friendly layout (contiguous 5.6KB
    runs per partition); the final [B,T,C,C] permutation happens on host.
"""

import sys

if "/opt/trn_rl_repo" not in sys.path:
    sys.path.insert(0, "/opt/trn_rl_repo")

import math
from contextlib import ExitStack

import ml_dtypes
import numpy as np

import concourse.bass as bass
import concourse.tile as tile
from concourse import bacc, mybir
from concourse.bass_utils import run_bass_kernel_spmd

# Problem constants
B, C, T = 4096, 22, 9
HID = 64
N_CORES = 8
B_LOC = B // N_CORES              # 512
POS = B_LOC * T                   # 4608 positions per core
NCOL = POS * C                    # 101376 columns per core

# Tiling
CHUNK_POS = 16                    # positions per chunk
CHUNK_COL = CHUNK_POS * C         # 352 columns
HALF = CHUNK_COL // 2             # 176 (fold half)
N_CHUNK = POS // CHUNK_POS        # 288
A_CHUNKS = 4                      # chunks per A-psum unit (softmax batch)
A_W = A_CHUNKS * 4 * C            # 352 floats per A unit
SB_CHUNKS = 16                    # chunks per superblock
SB_COL = SB_CHUNKS * CHUNK_COL    # 5632 columns
N_SB = N_CHUNK // SB_CHUNKS       # 18
OUT_W = SB_CHUNKS * 4 * C         # 1408 out floats per partition per sb
SCALE = 1.0 / math.sqrt(float(C))

BF16 = mybir.dt.bfloat16
F32 = mybir.dt.float32


def build_program():
    nc = bacc.Bacc("TRN2", target_bir_lowering=False, debug=False,
                   num_devices=N_CORES)

    # folded freq: [128, NCOL/2]
    xf = nc.dram_tensor("xf", [128, NCOL // 2], BF16, kind="ExternalInput")
    xt = nc.dram_tensor("xt", [2 * HID, NCOL], BF16, kind="ExternalInput")
    # wf_fold: 4 blocks of 64 cols: [W1T;0],[0;W1T],[W2T;0],[0;W2T]
    wf = nc.dram_tensor("wf", [128, 4 * HID], BF16, kind="ExternalInput")
    wt = nc.dram_tensor("wt", [2 * HID, 2 * HID], BF16, kind="ExternalInput")
    bias1 = nc.dram_tensor("bias1", [2 * HID, 1], F32, kind="ExternalInput")
    bias2 = nc.dram_tensor("bias2", [2 * HID, 1], F32, kind="ExternalInput")
    # device-layout output: [q4, cc, sb, (chunk, qg, e)]
    y = nc.dram_tensor("y", [4, C, N_SB, OUT_W], F32, kind="ExternalOutput")

    with tile.TileContext(nc, trace_sim=False) as tc, ExitStack() as ctx:
        pin_f = ctx.enter_context(tc.tile_pool(name="pin_f", bufs=2))
        pin_t = ctx.enter_context(tc.tile_pool(name="pin_t", bufs=2))
        pout = ctx.enter_context(tc.tile_pool(name="pout", bufs=3))
        pg = ctx.enter_context(tc.tile_pool(name="pg", bufs=5))
        psm = ctx.enter_context(tc.tile_pool(name="psm", bufs=5))
        pps_g = ctx.enter_context(tc.tile_pool(name="pps_g", bufs=3, space="PSUM"))
        pps_a = ctx.enter_context(tc.tile_pool(name="pps_a", bufs=2, space="PSUM"))
        singles = ctx.enter_context(tc.tile_pool(name="singles", bufs=1))

        wf_sb = singles.tile([128, 4 * HID], BF16)
        nc.sync.dma_start(out=wf_sb[:], in_=wf.ap())
        wt_sb = singles.tile([2 * HID, 2 * HID], BF16)
        nc.sync.dma_start(out=wt_sb[:], in_=wt.ap())
        b1_sb = singles.tile([2 * HID, 1], F32)
        nc.sync.dma_start(out=b1_sb[:], in_=bias1.ap())
        b2_sb = singles.tile([2 * HID, 1], F32)
        nc.sync.dma_start(out=b2_sb[:], in_=bias2.ap())

        xf_ap = xf.ap()
        xt_ap = xt.ap()
        y_ap = y.ap()

        # software-pipelined emission: stage1(g+1) is emitted before
        # stage2(g) so the PE never sits in a scheduled-order stall
        # waiting for the evacuation of chunk g.
        live = {}
        sb_tiles = {}
        out_tiles = {}

        def emit_stage1(g):
            s, ci = divmod(g, SB_CHUNKS)
            if ci == 0:
                xf_t = pin_f.tile([128, SB_COL // 2], BF16)
                nc.sync.dma_start(
                    out=xf_t[:],
                    in_=xf_ap[:, s * SB_COL // 2:(s + 1) * SB_COL // 2])
                xt_t = pin_t.tile([2 * HID, SB_COL], BF16)
                nc.sync.dma_start(
                    out=xt_t[:], in_=xt_ap[:, s * SB_COL:(s + 1) * SB_COL])
                sb_tiles[s] = (xf_t, xt_t)
            xf_t, xt_t = sb_tiles[s]
            xfold = xf_t[:, ci * HALF:(ci + 1) * HALF]
            xt_s = xt_t[:, ci * CHUNK_COL:(ci + 1) * CHUNK_COL]

            ps_g1 = pps_g.tile([128, CHUNK_COL], F32, tag="g1")
            ps_g2 = pps_g.tile([128, CHUNK_COL], F32, tag="g2")
            # All matmuls are tile_size (128, 32); issue round-robin across
            # the four column-tiles so they execute concurrently.
            nc.tensor.matmul(ps_g1[0:32, 0:HALF], wf_sb[:, 0:32], xfold)
            nc.tensor.matmul(ps_g1[32:64, 0:HALF], wf_sb[:, 32:64], xfold)
            nc.tensor.matmul(ps_g1[64:96, :], wt_sb[:, 0:32], xt_s)
            nc.tensor.matmul(ps_g1[96:128, :], wt_sb[:, 32:64], xt_s,
                             tile_position=(0, 96))
            nc.tensor.matmul(ps_g1[0:32, HALF:2 * HALF], wf_sb[:, 64:96], xfold)
            nc.tensor.matmul(ps_g1[32:64, HALF:2 * HALF], wf_sb[:, 96:128], xfold)
            nc.tensor.matmul(ps_g2[64:96, :], wt_sb[:, 64:96], xt_s)
            nc.tensor.matmul(ps_g2[96:128, :], wt_sb[:, 96:128], xt_s,
                             tile_position=(0, 96))
            nc.tensor.matmul(ps_g2[0:32, 0:HALF], wf_sb[:, 128:160], xfold)
            nc.tensor.matmul(ps_g2[32:64, 0:HALF], wf_sb[:, 160:192], xfold)
            nc.tensor.matmul(ps_g2[0:32, HALF:2 * HALF], wf_sb[:, 192:224], xfold)
            nc.tensor.matmul(ps_g2[32:64, HALF:2 * HALF], wf_sb[:, 224:256], xfold)

            g1 = pg.tile([128, CHUNK_COL], BF16, tag="g1s")
            nc.scalar.activation(
                g1[:], ps_g1[:], mybir.ActivationFunctionType.Relu,
                bias=b1_sb[:], scale=1.0,
            )
            g2 = pg.tile([128, CHUNK_COL], BF16, tag="g2s")
            nc.vector.tensor_scalar(
                out=g2[:], in0=ps_g2[:], scalar1=b2_sb[:], scalar2=0.0,
                op0=mybir.AluOpType.add, op1=mybir.AluOpType.max,
            )
            live[g] = (g1, g2)

        def emit_stage2(g):
            s, ci = divmod(g, SB_CHUNKS)
            if ci == 0:
                out_tiles[s] = pout.tile([128, OUT_W], F32, tag="out_t", name="out_t")
            out_t = out_tiles[s]
            g1, g2 = live.pop(g)

            ca = ci % A_CHUNKS
            if ca == 0:
                live["A"] = pps_a.tile([128, A_W], F32, tag="A", name="ps_a")
            ps_a = live["A"]
            for q in range(CHUNK_POS):
                q4, qg = q % 4, q // 4 + 4 * ca
                nc.tensor.matmul(
                    ps_a[32 * q4:32 * q4 + C, C * qg:C * qg + C],
                    g1[:, C * q:C * q + C],
                    g2[:, C * q:C * q + C],
                    tile_position=(0, 32 * q4),
                )

            if ca == A_CHUNKS - 1:
                np_used = 3 * 32 + C  # 118 partitions carry real rows
                ngrp = 4 * A_CHUNKS
                e_sb = psm.tile([128, A_W], F32, tag="e")
                nc.scalar.activation(
                    e_sb[0:np_used, :], ps_a[0:np_used, :],
                    mybir.ActivationFunctionType.Exp, scale=SCALE,
                )
                e_v = e_sb[0:np_used, :].rearrange("p (g e) -> p g e", e=C)
                s_sb = psm.tile([128, ngrp], F32, tag="s")
                nc.vector.reduce_sum(s_sb[0:np_used, :], e_v,
                                     axis=mybir.AxisListType.X)
                r_sb = psm.tile([128, ngrp], F32, tag="r")
                nc.vector.reciprocal(r_sb[0:np_used, :], s_sb[0:np_used, :])

                o_v = out_t[0:np_used,
                            (ci + 1 - A_CHUNKS) * 4 * C:(ci + 1) * 4 * C
                            ].rearrange("p (g e) -> p g e", e=C)
                r_v = r_sb[0:np_used, :].unsqueeze(2)
                e_b, r_b = bass.broadcast_tensor_aps(e_v, r_v)
                nc.gpsimd.tensor_tensor(
                    out=o_v, in0=e_b, in1=r_b, op=mybir.AluOpType.mult)

            if ci == SB_CHUNKS - 1:
                for q4 in range(4):
                    nc.sync.dma_start(
                        out=y_ap[q4, :, s, :],
                        in_=out_t[32 * q4:32 * q4 + C, :],
                    )

        TOTAL = N_SB * SB_CHUNKS
        for g in range(TOTAL + 1):
            if g < TOTAL:
                emit_stage1(g)
            if g >= 1:
                emit_stage2(g - 1)

    nc.compile()
    return nc


def shard_inputs(inputs):
    freq = np.asarray(inputs["freq_feat"], dtype=np.float32)
    time = np.asarray(inputs["time_out"], dtype=np.float32)
    W1 = np.asarray(inputs["W1"], dtype=np.float32)
    W2 = np.asarray(inputs["W2"], dtype=np.float32)
    W3 = np.asarray(inputs["W3"], dtype=np.float32)
    W4 = np.asarray(inputs["W4"], dtype=np.float32)
    b1 = np.asarray(inputs["b1"], dtype=np.float32)
    b2 = np.asarray(inputs["b2"], dtype=np.float32)
    b3 = np.asarray(inputs["b3"], dtype=np.float32)
    b4 = np.asarray(inputs["b4"], dtype=np.float32)

    bf16 = ml_dtypes.bfloat16
    Zh = np.zeros((HID, HID // 2), dtype=np.float32)
    # wf_fold blocks (each [128, 32]):
    # [W1Tlo;0],[W1Thi;0],[0;W1Tlo],[0;W1Thi],[W2Tlo;0],[W2Thi;0],[0;W2Tlo],[0;W2Thi]
    def blk(Wt, half, top):
        w = Wt[:, 32 * half:32 * (half + 1)]
        return np.concatenate([w, Zh] if top else [Zh, w], axis=0)
    wf = np.concatenate([
        blk(W1.T, 0, True), blk(W1.T, 1, True),
        blk(W1.T, 0, False), blk(W1.T, 1, False),
        blk(W2.T, 0, True), blk(W2.T, 1, True),
        blk(W2.T, 0, False), blk(W2.T, 1, False),
    ], axis=1).astype(bf16)                                     # [128, 256]
    wt = np.concatenate([W3.T, W4.T], axis=1).astype(bf16)      # [128, 128]
    bias1 = np.concatenate([b1, b3]).reshape(128, 1).astype(np.float32)
    bias2 = np.concatenate([b2, b4]).reshape(128, 1).astype(np.float32)

    in_maps = []
    for i in range(N_CORES):
        fs = freq[B_LOC * i:B_LOC * (i + 1)]   # [512, 22, 64, 9]
        ts = time[B_LOC * i:B_LOC * (i + 1)]   # [512, 22, 128, 9]
        # [B, C, D, T] -> [D, B, T, C] -> [D, B*T*C]
        xf2 = fs.transpose(2, 0, 3, 1).reshape(HID, NCOL)
        xt2 = ts.transpose(2, 0, 3, 1).reshape(2 * HID, NCOL).astype(bf16)
        # fold: chunk-half pairing -> [128, NCOL/2]
        xf3 = xf2.reshape(HID, N_CHUNK, 2, HALF)
        xfold = np.concatenate([xf3[:, :, 0, :], xf3[:, :, 1, :]],
                               axis=0).reshape(128, NCOL // 2).astype(bf16)
        in_maps.append({
            "xf": np.ascontiguousarray(xfold),
            "xt": np.ascontiguousarray(xt2),
            "wf": wf, "wt": wt, "bias1": bias1, "bias2": bias2,
        })
    return in_maps


def unshard_output(y_dev):
    """[4, 22, 18, 1408] device layout -> [512, 9, 22, 22]."""
    a = y_dev.reshape(4, C, N_SB, SB_CHUNKS, 4, C)   # [q4, cc, s, ch, qg, e]
    a = a.transpose(2, 3, 4, 0, 1, 5)                # [s, ch, qg, q4, cc, e]
    return np.ascontiguousarray(a).reshape(B_LOC, T, C, C)


_NC_CACHE = None


def _get_program():
    global _NC_CACHE
    if _NC_CACHE is None:
        _NC_CACHE = build_program()
    return _NC_CACHE


def run(inputs, trace=False):
    nc = _get_program()
    in_maps = shard_inputs(inputs)
    res = run_bass_kernel_spmd(nc, in_maps, core_ids=list(range(N_CORES)),
                               trace=trace)
    outs = [unshard_output(np.asarray(r["y"], dtype=np.float32))
            for r in res.results]
    full = np.concatenate(outs, axis=0)  # [4096, 9, 22, 22]
    return full, res


def kernel(**inputs) -> np.ndarray:
    full, _ = run(inputs, trace=False)
    return full


# revision 17
# speedup vs baseline: 1.0677x; 1.0677x over previous
"""Trainium2 Bass kernel for AdjacencyMatrixLearning.

Computes, per (b, t):
    f1 = relu(freq @ W1.T + b1); f2 = relu(freq @ W2.T + b2)   # freq: [C, 64]
    t1 = relu(time @ W3.T + b3); t2 = relu(time @ W4.T + b4)   # time: [C, 128]
    A  = (f1 @ f2.T + t1 @ t2.T) / sqrt(C)
    out = softmax(A, axis=-1)                                   # [C, C]

Sharding: pure data-parallel over B across 8 NeuronCores.

Device-side strategy (per core, B_loc=512, T=9, C=22 -> 4608 positions):
  - Inputs pre-transposed on host to x[d, (b, t, c)], bf16.
  - freq is "folded" to K=128: column j of a chunk pairs with column j+176,
    stacked along partitions; weights are zero-padded block-diagonal so every
    stage-1 matmul is tile_size (128, 64). The f-matmuls run on column-tile
    (0,0) (psum partitions 0-63), the t-matmuls on (0,64), concurrently,
    producing g1 = [f1; t1], g2 = [f2; t2] directly.
  - Stage 2: per-position K=128 matmul A = g1_blk.T @ g2_blk -> PSUM [22, 22]
    at partition base 32*(q%4) so softmax access patterns stay affine.
    A-psum units span 4 chunks to amortize softmax instruction overheads.
  - Softmax without max-subtraction (logits bounded): exp on ScalarE with the
    1/sqrt(C) scale folded in, sum+recip on VectorE, multiply on GpSimd.
  - Output is written to DRAM in an SBUF-friendly layout (contiguous 5.6KB
    runs per partition); the final [B,T,C,C] permutation happens on host.
"""

import sys

if "/opt/trn_rl_repo" not in sys.path:
    sys.path.insert(0, "/opt/trn_rl_repo")

import math
from contextlib import ExitStack

import ml_dtypes
import numpy as np

import concourse.bass as bass
import concourse.tile as tile
from concourse import bacc, mybir
from concourse.bass_utils import run_bass_kernel_spmd

# Problem constants
B, C, T = 4096, 22, 9
HID = 64
N_CORES = 8
B_LOC = B // N_CORES              # 512
POS = B_LOC * T                   # 4608 positions per core
NCOL = POS * C                    # 101376 columns per core

# Tiling
CHUNK_POS = 16                    # positions per chunk
CHUNK_COL = CHUNK_POS * C         # 352 columns
HALF = CHUNK_COL // 2             # 176 (fold half)
N_CHUNK = POS // CHUNK_POS        # 288
A_CHUNKS = 4                      # chunks per A-psum unit (softmax batch)
A_W = A_CHUNKS * 4 * C            # 352 floats per A unit
SB_CHUNKS = 16                    # chunks per superblock
SB_COL = SB_CHUNKS * CHUNK_COL    # 5632 columns
N_SB = N_CHUNK // SB_CHUNKS       # 18
OUT_W = SB_CHUNKS * 4 * C         # 1408 out floats per partition per sb
SCALE = 1.0 / math.sqrt(float(C))

BF16 = mybir.dt.bfloat16
F32 = mybir.dt.float32


def build_program():
    nc = bacc.Bacc("TRN2", target_bir_lowering=False, debug=False,
                   num_devices=N_CORES)

    # folded freq: [128, NCOL/2]
    xf = nc.dram_tensor("xf", [128, NCOL // 2], BF16, kind="ExternalInput")
    xt = nc.dram_tensor("xt", [2 * HID, NCOL], BF16, kind="ExternalInput")
    # wf_fold: 4 blocks of 64 cols: [W1T;0],[0;W1T],[W2T;0],[0;W2T]
    wf = nc.dram_tensor("wf", [128, 4 * HID], BF16, kind="ExternalInput")
    wt = nc.dram_tensor("wt", [2 * HID, 2 * HID], BF16, kind="ExternalInput")
    bias1 = nc.dram_tensor("bias1", [2 * HID, 1], F32, kind="ExternalInput")
    bias2 = nc.dram_tensor("bias2", [2 * HID, 1], F32, kind="ExternalInput")
    # device-layout output: [q4, cc, sb, (chunk, qg, e)]
    y = nc.dram_tensor("y", [4, C, N_SB, OUT_W], F32, kind="ExternalOutput")

    with tile.TileContext(nc, trace_sim=False) as tc, ExitStack() as ctx:
        pin_f = ctx.enter_context(tc.tile_pool(name="pin_f", bufs=2))
        pin_t = ctx.enter_context(tc.tile_pool(name="pin_t", bufs=2))
        pout = ctx.enter_context(tc.tile_pool(name="pout", bufs=3))
        pg = ctx.enter_context(tc.tile_pool(name="pg", bufs=5))
        psm = ctx.enter_context(tc.tile_pool(name="psm", bufs=5))
        pps_g = ctx.enter_context(tc.tile_pool(name="pps_g", bufs=3, space="PSUM"))
        pps_a = ctx.enter_context(tc.tile_pool(name="pps_a", bufs=2, space="PSUM"))
        singles = ctx.enter_context(tc.tile_pool(name="singles", bufs=1))

        wf_sb = singles.tile([128, 4 * HID], BF16)
        nc.sync.dma_start(out=wf_sb[:], in_=wf.ap())
        wt_sb = singles.tile([2 * HID, 2 * HID], BF16)
        nc.sync.dma_start(out=wt_sb[:], in_=wt.ap())
        b1_sb = singles.tile([2 * HID, 1], F32)
        nc.sync.dma_start(out=b1_sb[:], in_=bias1.ap())
        b2_sb = singles.tile([2 * HID, 1], F32)
        nc.sync.dma_start(out=b2_sb[:], in_=bias2.ap())

        xf_ap = xf.ap()
        xt_ap = xt.ap()
        y_ap = y.ap()

        # software-pipelined emission: stage1(g+1) is emitted before
        # stage2(g) so the PE never sits in a scheduled-order stall
        # waiting for the evacuation of chunk g.
        live = {}
        sb_tiles = {}
        out_tiles = {}

        def emit_stage1(g):
            s, ci = divmod(g, SB_CHUNKS)
            if ci == 0:
                xf_t = pin_f.tile([128, SB_COL // 2], BF16)
                nc.sync.dma_start(
                    out=xf_t[:],
                    in_=xf_ap[:, s * SB_COL // 2:(s + 1) * SB_COL // 2])
                xt_t = pin_t.tile([2 * HID, SB_COL], BF16)
                nc.sync.dma_start(
                    out=xt_t[:], in_=xt_ap[:, s * SB_COL:(s + 1) * SB_COL])
                sb_tiles[s] = (xf_t, xt_t)
            xf_t, xt_t = sb_tiles[s]
            xfold = xf_t[:, ci * HALF:(ci + 1) * HALF]
            xt_s = xt_t[:, ci * CHUNK_COL:(ci + 1) * CHUNK_COL]

            ps_g1 = pps_g.tile([128, CHUNK_COL], F32, tag="g1")
            ps_g2 = pps_g.tile([128, CHUNK_COL], F32, tag="g2")
            nc.tensor.matmul(ps_g1[0:64, 0:HALF], wf_sb[:, 0:64], xfold)
            nc.tensor.matmul(ps_g1[0:64, HALF:2 * HALF], wf_sb[:, 64:128], xfold)
            nc.tensor.matmul(ps_g2[0:64, 0:HALF], wf_sb[:, 128:192], xfold)
            nc.tensor.matmul(ps_g2[0:64, HALF:2 * HALF], wf_sb[:, 192:256], xfold)
            nc.tensor.matmul(ps_g1[64:128, :], wt_sb[:, 0:64], xt_s)
            nc.tensor.matmul(ps_g2[64:128, :], wt_sb[:, 64:128], xt_s)

            g1 = pg.tile([128, CHUNK_COL], BF16, tag="g1s")
            nc.scalar.activation(
                g1[:], ps_g1[:], mybir.ActivationFunctionType.Relu,
                bias=b1_sb[:], scale=1.0,
            )
            g2 = pg.tile([128, CHUNK_COL], BF16, tag="g2s")
            nc.vector.tensor_scalar(
                out=g2[:], in0=ps_g2[:], scalar1=b2_sb[:], scalar2=0.0,
                op0=mybir.AluOpType.add, op1=mybir.AluOpType.max,
            )
            live[g] = (g1, g2)

        def emit_stage2(g):
            s, ci = divmod(g, SB_CHUNKS)
            if ci == 0:
                out_tiles[s] = pout.tile([128, OUT_W], F32, tag="out_t", name="out_t")
            out_t = out_tiles[s]
            g1, g2 = live.pop(g)

            ca = ci % A_CHUNKS
            if ca == 0:
                live["A"] = pps_a.tile([128, A_W], F32, tag="A", name="ps_a")
            ps_a = live["A"]
            for q in range(CHUNK_POS):
                q4, qg = q % 4, q // 4 + 4 * ca
                nc.tensor.matmul(
                    ps_a[32 * q4:32 * q4 + C, C * qg:C * qg + C],
                    g1[:, C * q:C * q + C],
                    g2[:, C * q:C * q + C],
                    tile_position=(0, 32 * q4),
                )

            if ca == A_CHUNKS - 1:
                np_used = 3 * 32 + C  # 118 partitions carry real rows
                ngrp = 4 * A_CHUNKS
                e_sb = psm.tile([128, A_W], F32, tag="e")
                nc.scalar.activation(
                    e_sb[0:np_used, :], ps_a[0:np_used, :],
                    mybir.ActivationFunctionType.Exp, scale=SCALE,
                )
                e_v = e_sb[0:np_used, :].rearrange("p (g e) -> p g e", e=C)
                s_sb = psm.tile([128, ngrp], F32, tag="s")
                nc.vector.reduce_sum(s_sb[0:np_used, :], e_v,
                                     axis=mybir.AxisListType.X)
                r_sb = psm.tile([128, ngrp], F32, tag="r")
                nc.vector.reciprocal(r_sb[0:np_used, :], s_sb[0:np_used, :])

                o_v = out_t[0:np_used,
                            (ci + 1 - A_CHUNKS) * 4 * C:(ci + 1) * 4 * C
                            ].rearrange("p (g e) -> p g e", e=C)
                r_v = r_sb[0:np_used, :].unsqueeze(2)
                e_b, r_b = bass.broadcast_tensor_aps(e_v, r_v)
                nc.gpsimd.tensor_tensor(
                    out=o_v, in0=e_b, in1=r_b, op=mybir.AluOpType.mult)

            if ci == SB_CHUNKS - 1:
                for q4 in range(4):
                    nc.sync.dma_start(
                        out=y_ap[q4, :, s, :],
                        in_=out_t[32 * q4:32 * q4 + C, :],
                    )

        TOTAL = N_SB * SB_CHUNKS
        for gp in range(0, TOTAL + 2, 2):
            if gp < TOTAL:
                emit_stage1(gp)
                emit_stage1(gp + 1)
            if gp >= 2:
                emit_stage2(gp - 2)
                emit_stage2(gp - 1)

    nc.compile()
    return nc


def shard_inputs(inputs):
    freq = np.asarray(inputs["freq_feat"], dtype=np.float32)
    time = np.asarray(inputs["time_out"], dtype=np.float32)
    W1 = np.asarray(inputs["W1"], dtype=np.float32)
    W2 = np.asarray(inputs["W2"], dtype=np.float32)
    W3 = np.asarray(inputs["W3"], dtype=np.float32)
    W4 = np.asarray(inputs["W4"], dtype=np.float32)
    b1 = np.asarray(inputs["b1"], dtype=np.float32)
    b2 = np.asarray(inputs["b2"], dtype=np.float32)
    b3 = np.asarray(inputs["b3"], dtype=np.float32)
    b4 = np.asarray(inputs["b4"], dtype=np.float32)

    bf16 = ml_dtypes.bfloat16
    Z = np.zeros((HID, HID), dtype=np.float32)
    # wf_fold blocks (each [128, 64]): [W1T;0], [0;W1T], [W2T;0], [0;W2T]
    wf = np.concatenate([
        np.concatenate([W1.T, Z], axis=0),
        np.concatenate([Z, W1.T], axis=0),
        np.concatenate([W2.T, Z], axis=0),
        np.concatenate([Z, W2.T], axis=0),
    ], axis=1).astype(bf16)                                     # [128, 256]
    wt = np.concatenate([W3.T, W4.T], axis=1).astype(bf16)      # [128, 128]
    bias1 = np.concatenate([b1, b3]).reshape(128, 1).astype(np.float32)
    bias2 = np.concatenate([b2, b4]).reshape(128, 1).astype(np.float32)

    in_maps = []
    for i in range(N_CORES):
        fs = freq[B_LOC * i:B_LOC * (i + 1)]   # [512, 22, 64, 9]
        ts = time[B_LOC * i:B_LOC * (i + 1)]   # [512, 22, 128, 9]
        # [B, C, D, T] -> [D, B, T, C] -> [D, B*T*C]
        xf2 = fs.transpose(2, 0, 3, 1).reshape(HID, NCOL)
        xt2 = ts.transpose(2, 0, 3, 1).reshape(2 * HID, NCOL).astype(bf16)
        # fold: chunk-half pairing -> [128, NCOL/2]
        xf3 = xf2.reshape(HID, N_CHUNK, 2, HALF)
        xfold = np.concatenate([xf3[:, :, 0, :], xf3[:, :, 1, :]],
                               axis=0).reshape(128, NCOL // 2).astype(bf16)
        in_maps.append({
            "xf": np.ascontiguousarray(xfold),
            "xt": np.ascontiguousarray(xt2),
            "wf": wf, "wt": wt, "bias1": bias1, "bias2": bias2,
        })
    return in_maps


def unshard_output(y_dev):
    """[4, 22, 18, 1408] device layout -> [512, 9, 22, 22]."""
    a = y_dev.reshape(4, C, N_SB, SB_CHUNKS, 4, C)   # [q4, cc, s, ch, qg, e]
    a = a.transpose(2, 3, 4, 0, 1, 5)                # [s, ch, qg, q4, cc, e]
    return np.ascontiguousarray(a).reshape(B_LOC, T, C, C)


_NC_CACHE = None


def _get_program():
    global _NC_CACHE
    if _NC_CACHE is None:
        _NC_CACHE = build_program()
    return _NC_CACHE


def run(inputs, trace=False):
    nc = _get_program()
    in_maps = shard_inputs(inputs)
    res = run_bass_kernel_spmd(nc, in_maps, core_ids=list(range(N_CORES)),
                               trace=trace)
    outs = [unshard_output(np.asarray(r["y"], dtype=np.float32))
            for r in res.results]
    full = np.concatenate(outs, axis=0)  # [4096, 9, 22, 22]
    return full, res


def kernel(**inputs) -> np.ndarray:
    full, _ = run(inputs, trace=False)
    return full


# revision 19
# speedup vs baseline: 1.0696x; 1.0018x over previous
"""Trainium2 Bass kernel for AdjacencyMatrixLearning.

Computes, per (b, t):
    f1 = relu(freq @ W1.T + b1); f2 = relu(freq @ W2.T + b2)   # freq: [C, 64]
    t1 = relu(time @ W3.T + b3); t2 = relu(time @ W4.T + b4)   # time: [C, 128]
    A  = (f1 @ f2.T + t1 @ t2.T) / sqrt(C)
    out = softmax(A, axis=-1)                                   # [C, C]

Sharding: pure data-parallel over B across 8 NeuronCores.

Device-side strategy (per core, B_loc=512, T=9, C=22 -> 4608 positions):
  - Inputs pre-transposed on host to x[d, (b, t, c)], bf16.
  - freq is "folded" to K=128: column j of a chunk pairs with column j+176,
    stacked along partitions; weights are zero-padded block-diagonal so every
    stage-1 matmul is tile_size (128, 64). The f-matmuls run on column-tile
    (0,0) (psum partitions 0-63), the t-matmuls on (0,64), concurrently,
    producing g1 = [f1; t1], g2 = [f2; t2] directly.
  - Stage 2: per-position K=128 matmul A = g1_blk.T @ g2_blk -> PSUM [22, 22]
    at partition base 32*(q%4) so softmax access patterns stay affine.
    A-psum units span 4 chunks to amortize softmax instruction overheads.
  - Softmax without max-subtraction (logits bounded): exp on ScalarE with the
    1/sqrt(C) scale folded in, sum+recip on VectorE, multiply on GpSimd.
  - Output is written to DRAM in an SBUF-friendly layout (contiguous 5.6KB
    runs per partition); the final [B,T,C,C] permutation happens on host.
"""

import sys

if "/opt/trn_rl_repo" not in sys.path:
    sys.path.insert(0, "/opt/trn_rl_repo")

import math
from contextlib import ExitStack

import ml_dtypes
import numpy as np

import concourse.bass as bass
import concourse.tile as tile
from concourse import bacc, mybir
from concourse.bass_utils import run_bass_kernel_spmd

# Problem constants
B, C, T = 4096, 22, 9
HID = 64
N_CORES = 8
B_LOC = B // N_CORES              # 512
POS = B_LOC * T                   # 4608 positions per core
NCOL = POS * C                    # 101376 columns per core

# Tiling
CHUNK_POS = 16                    # positions per chunk
CHUNK_COL = CHUNK_POS * C         # 352 columns
HALF = CHUNK_COL // 2             # 176 (fold half)
N_CHUNK = POS // CHUNK_POS        # 288
A_CHUNKS = 4                      # chunks per A-psum unit (softmax batch)
A_W = A_CHUNKS * 4 * C            # 352 floats per A unit
SB_CHUNKS = 16                    # chunks per superblock
SB_COL = SB_CHUNKS * CHUNK_COL    # 5632 columns
N_SB = N_CHUNK // SB_CHUNKS       # 18
OUT_W = SB_CHUNKS * 4 * C         # 1408 out floats per partition per sb
SCALE = 1.0 / math.sqrt(float(C))

BF16 = mybir.dt.bfloat16
F32 = mybir.dt.float32


def build_program():
    nc = bacc.Bacc("TRN2", target_bir_lowering=False, debug=False,
                   num_devices=N_CORES)

    # folded freq: [128, NCOL/2]
    xf = nc.dram_tensor("xf", [128, NCOL // 2], BF16, kind="ExternalInput")
    xt = nc.dram_tensor("xt", [2 * HID, NCOL], BF16, kind="ExternalInput")
    # wf_fold: 4 blocks of 64 cols: [W1T;0],[0;W1T],[W2T;0],[0;W2T]
    wf = nc.dram_tensor("wf", [128, 4 * HID], BF16, kind="ExternalInput")
    wt = nc.dram_tensor("wt", [2 * HID, 2 * HID], BF16, kind="ExternalInput")
    bias1 = nc.dram_tensor("bias1", [2 * HID, 1], F32, kind="ExternalInput")
    bias2 = nc.dram_tensor("bias2", [2 * HID, 1], F32, kind="ExternalInput")
    # device-layout output: [q4, cc, sb, (chunk, qg, e)]
    y = nc.dram_tensor("y", [4, C, N_SB, OUT_W], F32, kind="ExternalOutput")

    with tile.TileContext(nc, trace_sim=False) as tc, ExitStack() as ctx:
        pin_f = ctx.enter_context(tc.tile_pool(name="pin_f", bufs=3))
        pin_t = ctx.enter_context(tc.tile_pool(name="pin_t", bufs=3))
        pout = ctx.enter_context(tc.tile_pool(name="pout", bufs=3))
        pg = ctx.enter_context(tc.tile_pool(name="pg", bufs=5))
        psm = ctx.enter_context(tc.tile_pool(name="psm", bufs=5))
        pps_g = ctx.enter_context(tc.tile_pool(name="pps_g", bufs=3, space="PSUM"))
        pps_a = ctx.enter_context(tc.tile_pool(name="pps_a", bufs=2, space="PSUM"))
        singles = ctx.enter_context(tc.tile_pool(name="singles", bufs=1))

        wf_sb = singles.tile([128, 4 * HID], BF16)
        nc.sync.dma_start(out=wf_sb[:], in_=wf.ap())
        wt_sb = singles.tile([2 * HID, 2 * HID], BF16)
        nc.sync.dma_start(out=wt_sb[:], in_=wt.ap())
        b1_sb = singles.tile([2 * HID, 1], F32)
        nc.sync.dma_start(out=b1_sb[:], in_=bias1.ap())
        b2_sb = singles.tile([2 * HID, 1], F32)
        nc.sync.dma_start(out=b2_sb[:], in_=bias2.ap())

        xf_ap = xf.ap()
        xt_ap = xt.ap()
        y_ap = y.ap()

        # software-pipelined emission: stage1(g+1) is emitted before
        # stage2(g) so the PE never sits in a scheduled-order stall
        # waiting for the evacuation of chunk g.
        live = {}
        sb_tiles = {}
        out_tiles = {}

        def emit_stage1(g):
            s, ci = divmod(g, SB_CHUNKS)
            if ci == 0:
                xf_t = pin_f.tile([128, SB_COL // 2], BF16)
                nc.sync.dma_start(
                    out=xf_t[:],
                    in_=xf_ap[:, s * SB_COL // 2:(s + 1) * SB_COL // 2])
                xt_t = pin_t.tile([2 * HID, SB_COL], BF16)
                nc.sync.dma_start(
                    out=xt_t[:], in_=xt_ap[:, s * SB_COL:(s + 1) * SB_COL])
                sb_tiles[s] = (xf_t, xt_t)
            xf_t, xt_t = sb_tiles[s]
            xfold = xf_t[:, ci * HALF:(ci + 1) * HALF]
            xt_s = xt_t[:, ci * CHUNK_COL:(ci + 1) * CHUNK_COL]

            ps_g1 = pps_g.tile([128, CHUNK_COL], F32, tag="g1")
            ps_g2 = pps_g.tile([128, CHUNK_COL], F32, tag="g2")
            nc.tensor.matmul(ps_g1[0:64, 0:HALF], wf_sb[:, 0:64], xfold)
            nc.tensor.matmul(ps_g1[0:64, HALF:2 * HALF], wf_sb[:, 64:128], xfold)
            nc.tensor.matmul(ps_g2[0:64, 0:HALF], wf_sb[:, 128:192], xfold)
            nc.tensor.matmul(ps_g2[0:64, HALF:2 * HALF], wf_sb[:, 192:256], xfold)
            nc.tensor.matmul(ps_g1[64:128, :], wt_sb[:, 0:64], xt_s)
            nc.tensor.matmul(ps_g2[64:128, :], wt_sb[:, 64:128], xt_s)

            g1 = pg.tile([128, CHUNK_COL], BF16, tag="g1s")
            nc.scalar.activation(
                g1[:], ps_g1[:], mybir.ActivationFunctionType.Relu,
                bias=b1_sb[:], scale=1.0,
            )
            g2 = pg.tile([128, CHUNK_COL], BF16, tag="g2s")
            nc.vector.tensor_scalar(
                out=g2[:], in0=ps_g2[:], scalar1=b2_sb[:], scalar2=0.0,
                op0=mybir.AluOpType.add, op1=mybir.AluOpType.max,
            )
            live[g] = (g1, g2)

        def emit_stage2(g):
            s, ci = divmod(g, SB_CHUNKS)
            if ci == 0:
                out_tiles[s] = pout.tile([128, OUT_W], F32, tag="out_t", name="out_t")
            out_t = out_tiles[s]
            g1, g2 = live.pop(g)

            ca = ci % A_CHUNKS
            if ca == 0:
                live["A"] = pps_a.tile([128, A_W], F32, tag="A", name="ps_a")
            ps_a = live["A"]
            for q in range(CHUNK_POS):
                q4, qg = q % 4, q // 4 + 4 * ca
                nc.tensor.matmul(
                    ps_a[32 * q4:32 * q4 + C, C * qg:C * qg + C],
                    g1[:, C * q:C * q + C],
                    g2[:, C * q:C * q + C],
                    tile_position=(0, 32 * q4),
                )

            if ca == A_CHUNKS - 1:
                np_used = 3 * 32 + C  # 118 partitions carry real rows
                ngrp = 4 * A_CHUNKS
                e_sb = psm.tile([128, A_W], F32, tag="e")
                nc.scalar.activation(
                    e_sb[0:np_used, :], ps_a[0:np_used, :],
                    mybir.ActivationFunctionType.Exp, scale=SCALE,
                )
                e_v = e_sb[0:np_used, :].rearrange("p (g e) -> p g e", e=C)
                s_sb = psm.tile([128, ngrp], F32, tag="s")
                nc.vector.reduce_sum(s_sb[0:np_used, :], e_v,
                                     axis=mybir.AxisListType.X)
                r_sb = psm.tile([128, ngrp], F32, tag="r")
                nc.vector.reciprocal(r_sb[0:np_used, :], s_sb[0:np_used, :])

                o_v = out_t[0:np_used,
                            (ci + 1 - A_CHUNKS) * 4 * C:(ci + 1) * 4 * C
                            ].rearrange("p (g e) -> p g e", e=C)
                r_v = r_sb[0:np_used, :].unsqueeze(2)
                e_b, r_b = bass.broadcast_tensor_aps(e_v, r_v)
                nc.gpsimd.tensor_tensor(
                    out=o_v, in0=e_b, in1=r_b, op=mybir.AluOpType.mult)

            if ci == SB_CHUNKS - 1:
                for q4 in range(4):
                    nc.sync.dma_start(
                        out=y_ap[q4, :, s, :],
                        in_=out_t[32 * q4:32 * q4 + C, :],
                    )

        TOTAL = N_SB * SB_CHUNKS
        for g in range(TOTAL + 2):
            if g < TOTAL:
                emit_stage1(g)
            if g >= 2:
                emit_stage2(g - 2)

    nc.compile()
    return nc


def shard_inputs(inputs):
    freq = np.asarray(inputs["freq_feat"], dtype=np.float32)
    time = np.asarray(inputs["time_out"], dtype=np.float32)
    W1 = np.asarray(inputs["W1"], dtype=np.float32)
    W2 = np.asarray(inputs["W2"], dtype=np.float32)
    W3 = np.asarray(inputs["W3"], dtype=np.float32)
    W4 = np.asarray(inputs["W4"], dtype=np.float32)
    b1 = np.asarray(inputs["b1"], dtype=np.float32)
    b2 = np.asarray(inputs["b2"], dtype=np.float32)
    b3 = np.asarray(inputs["b3"], dtype=np.float32)
    b4 = np.asarray(inputs["b4"], dtype=np.float32)

    bf16 = ml_dtypes.bfloat16
    Z = np.zeros((HID, HID), dtype=np.float32)
    # wf_fold blocks (each [128, 64]): [W1T;0], [0;W1T], [W2T;0], [0;W2T]
    wf = np.concatenate([
        np.concatenate([W1.T, Z], axis=0),
        np.concatenate([Z, W1.T], axis=0),
        np.concatenate([W2.T, Z], axis=0),
        np.concatenate([Z, W2.T], axis=0),
    ], axis=1).astype(bf16)                                     # [128, 256]
    wt = np.concatenate([W3.T, W4.T], axis=1).astype(bf16)      # [128, 128]
    bias1 = np.concatenate([b1, b3]).reshape(128, 1).astype(np.float32)
    bias2 = np.concatenate([b2, b4]).reshape(128, 1).astype(np.float32)

    in_maps = []
    for i in range(N_CORES):
        fs = freq[B_LOC * i:B_LOC * (i + 1)]   # [512, 22, 64, 9]
        ts = time[B_LOC * i:B_LOC * (i + 1)]   # [512, 22, 128, 9]
        # [B, C, D, T] -> [D, B, T, C] -> [D, B*T*C]
        xf2 = fs.transpose(2, 0, 3, 1).reshape(HID, NCOL)
        xt2 = ts.transpose(2, 0, 3, 1).reshape(2 * HID, NCOL).astype(bf16)
        # fold: chunk-half pairing -> [128, NCOL/2]
        xf3 = xf2.reshape(HID, N_CHUNK, 2, HALF)
        xfold = np.concatenate([xf3[:, :, 0, :], xf3[:, :, 1, :]],
                               axis=0).reshape(128, NCOL // 2).astype(bf16)
        in_maps.append({
            "xf": np.ascontiguousarray(xfold),
            "xt": np.ascontiguousarray(xt2),
            "wf": wf, "wt": wt, "bias1": bias1, "bias2": bias2,
        })
    return in_maps


def unshard_output(y_dev):
    """[4, 22, 18, 1408] device layout -> [512, 9, 22, 22]."""
    a = y_dev.reshape(4, C, N_SB, SB_CHUNKS, 4, C)   # [q4, cc, s, ch, qg, e]
    a = a.transpose(2, 3, 4, 0, 1, 5)                # [s, ch, qg, q4, cc, e]
    return np.ascontiguousarray(a).reshape(B_LOC, T, C, C)


_NC_CACHE = None


def _get_program():
    global _NC_CACHE
    if _NC_CACHE is None:
        _NC_CACHE = build_program()
    return _NC_CACHE


def run(inputs, trace=False):
    nc = _get_program()
    in_maps = shard_inputs(inputs)
    res = run_bass_kernel_spmd(nc, in_maps, core_ids=list(range(N_CORES)),
                               trace=trace)
    outs = [unshard_output(np.asarray(r["y"], dtype=np.float32))
            for r in res.results]
    full = np.concatenate(outs, axis=0)  # [4096, 9, 22, 22]
    return full, res


def kernel(**inputs) -> np.ndarray:
    full, _ = run(inputs, trace=False)
    return full


# revision 20
# speedup vs baseline: 1.2014x; 1.1233x over previous
"""Trainium2 Bass kernel for AdjacencyMatrixLearning.

Computes, per (b, t):
    f1 = relu(freq @ W1.T + b1); f2 = relu(freq @ W2.T + b2)   # freq: [C, 64]
    t1 = relu(time @ W3.T + b3); t2 = relu(time @ W4.T + b4)   # time: [C, 128]
    A  = (f1 @ f2.T + t1 @ t2.T) / sqrt(C)
    out = softmax(A, axis=-1)                                   # [C, C]

Sharding: pure data-parallel over B across 8 NeuronCores.

Device-side strategy (per core, B_loc=512, T=9, C=22 -> 4608 positions):
  - Inputs pre-transposed on host to x[d, (b, t, c)], bf16.
  - freq is "folded" to K=128: column j of a chunk pairs with column j+176,
    stacked along partitions; weights are zero-padded block-diagonal so every
    stage-1 matmul is tile_size (128, 64). The f-matmuls run on column-tile
    (0,0) (psum partitions 0-63), the t-matmuls on (0,64), concurrently,
    producing g1 = [f1; t1], g2 = [f2; t2] directly.
  - Stage 2: per-position K=128 matmul A = g1_blk.T @ g2_blk -> PSUM [22, 22]
    at partition base 32*(q%4) so softmax access patterns stay affine.
    A-psum units span 4 chunks to amortize softmax instruction overheads.
  - Softmax without max-subtraction (logits bounded): exp on ScalarE with the
    1/sqrt(C) scale folded in, sum+recip on VectorE, multiply on GpSimd.
  - Output is written to DRAM in an SBUF-friendly layout (contiguous 5.6KB
    runs per partition); the final [B,T,C,C] permutation happens on host.
"""

import sys

if "/opt/trn_rl_repo" not in sys.path:
    sys.path.insert(0, "/opt/trn_rl_repo")

import math
from contextlib import ExitStack

import ml_dtypes
import numpy as np

import concourse.bass as bass
import concourse.tile as tile
from concourse import bacc, mybir
from concourse.bass_utils import run_bass_kernel_spmd

# Problem constants
B, C, T = 4096, 22, 9
HID = 64
N_CORES = 8
B_LOC = B // N_CORES              # 512
POS = B_LOC * T                   # 4608 positions per core
NCOL = POS * C                    # 101376 columns per core

# Tiling
CHUNK_POS = 16                    # positions per chunk
CHUNK_COL = CHUNK_POS * C         # 352 columns
HALF = CHUNK_COL // 2             # 176 (fold half)
N_CHUNK = POS // CHUNK_POS        # 288
A_CHUNKS = 4                      # chunks per A-psum unit (softmax batch)
A_W = A_CHUNKS * 4 * C            # 352 floats per A unit
SB_CHUNKS = 16                    # chunks per superblock
SB_COL = SB_CHUNKS * CHUNK_COL    # 5632 columns
N_SB = N_CHUNK // SB_CHUNKS       # 18
OUT_W = SB_CHUNKS * 4 * C         # 1408 out floats per partition per sb
SCALE = 1.0 / math.sqrt(float(C))

BF16 = mybir.dt.bfloat16
F32 = mybir.dt.float32


def build_program():
    nc = bacc.Bacc("TRN2", target_bir_lowering=False, debug=False,
                   num_devices=N_CORES)

    # folded freq: [128, NCOL/2]
    xf = nc.dram_tensor("xf", [128, NCOL // 2], BF16, kind="ExternalInput")
    xt = nc.dram_tensor("xt", [2 * HID, NCOL], BF16, kind="ExternalInput")
    # wf_fold: 4 blocks of 64 cols: [W1T;0],[0;W1T],[W2T;0],[0;W2T]
    wf = nc.dram_tensor("wf", [128, 4 * HID], BF16, kind="ExternalInput")
    wt = nc.dram_tensor("wt", [2 * HID, 2 * HID], BF16, kind="ExternalInput")
    bias1 = nc.dram_tensor("bias1", [2 * HID, 1], F32, kind="ExternalInput")
    bias2 = nc.dram_tensor("bias2", [2 * HID, 1], F32, kind="ExternalInput")
    # device-layout output: [q4, cc, sb, (chunk, qg, e)]
    y = nc.dram_tensor("y", [4, C, N_SB, OUT_W], F32, kind="ExternalOutput")

    with tile.TileContext(nc, trace_sim=False) as tc, ExitStack() as ctx:
        pin_f = ctx.enter_context(tc.tile_pool(name="pin_f", bufs=2))
        pin_t = ctx.enter_context(tc.tile_pool(name="pin_t", bufs=2))
        pout = ctx.enter_context(tc.tile_pool(name="pout", bufs=3))
        pg = ctx.enter_context(tc.tile_pool(name="pg", bufs=5))
        psm = ctx.enter_context(tc.tile_pool(name="psm", bufs=5))
        pps_g = ctx.enter_context(tc.tile_pool(name="pps_g", bufs=3, space="PSUM"))
        pps_a = ctx.enter_context(tc.tile_pool(name="pps_a", bufs=2, space="PSUM"))
        singles = ctx.enter_context(tc.tile_pool(name="singles", bufs=1))

        wf_sb = singles.tile([128, 4 * HID], BF16)
        nc.sync.dma_start(out=wf_sb[:], in_=wf.ap())
        wt_sb = singles.tile([2 * HID, 2 * HID], BF16)
        nc.sync.dma_start(out=wt_sb[:], in_=wt.ap())
        b1_sb = singles.tile([2 * HID, 1], F32)
        nc.sync.dma_start(out=b1_sb[:], in_=bias1.ap())
        b2_sb = singles.tile([2 * HID, 1], F32)
        nc.sync.dma_start(out=b2_sb[:], in_=bias2.ap())

        xf_ap = xf.ap()
        xt_ap = xt.ap()
        y_ap = y.ap()

        # software-pipelined emission: stage1(g+1) is emitted before
        # stage2(g) so the PE never sits in a scheduled-order stall
        # waiting for the evacuation of chunk g.
        live = {}
        sb_tiles = {}
        out_tiles = {}

        def emit_stage1(g):
            s, ci = divmod(g, SB_CHUNKS)
            if ci == 0:
                xf_t = pin_f.tile([128, SB_COL // 2], BF16)
                nc.sync.dma_start(
                    out=xf_t[:],
                    in_=xf_ap[:, s * SB_COL // 2:(s + 1) * SB_COL // 2])
                xt_t = pin_t.tile([2 * HID, SB_COL], BF16)
                nc.sync.dma_start(
                    out=xt_t[:], in_=xt_ap[:, s * SB_COL:(s + 1) * SB_COL])
                sb_tiles[s] = (xf_t, xt_t)
            xf_t, xt_t = sb_tiles[s]
            xfold = xf_t[:, ci * HALF:(ci + 1) * HALF]
            xt_s = xt_t[:, ci * CHUNK_COL:(ci + 1) * CHUNK_COL]

            ps_g1 = pps_g.tile([128, CHUNK_COL], F32, tag="g1")
            ps_g2 = pps_g.tile([128, CHUNK_COL], F32, tag="g2")
            nc.tensor.matmul(ps_g1[0:64, 0:HALF], wf_sb[:, 0:64], xfold)
            nc.tensor.matmul(ps_g1[0:64, HALF:2 * HALF], wf_sb[:, 64:128], xfold)
            nc.tensor.matmul(ps_g2[0:64, 0:HALF], wf_sb[:, 128:192], xfold)
            nc.tensor.matmul(ps_g2[0:64, HALF:2 * HALF], wf_sb[:, 192:256], xfold)
            nc.tensor.matmul(ps_g1[64:128, :], wt_sb[:, 0:64], xt_s)
            nc.tensor.matmul(ps_g2[64:128, :], wt_sb[:, 64:128], xt_s)

            g1 = pg.tile([128, CHUNK_COL], BF16, tag="g1s")
            nc.scalar.activation(
                g1[:], ps_g1[:], mybir.ActivationFunctionType.Relu,
                bias=b1_sb[:], scale=1.0,
            )
            g2 = pg.tile([128, CHUNK_COL], BF16, tag="g2s")
            nc.vector.tensor_scalar(
                out=g2[:], in0=ps_g2[:], scalar1=b2_sb[:], scalar2=0.0,
                op0=mybir.AluOpType.add, op1=mybir.AluOpType.max,
            )
            live[g] = (g1, g2)

        def emit_stage2(g):
            s, ci = divmod(g, SB_CHUNKS)
            if ci == 0:
                out_tiles[s] = pout.tile([128, OUT_W], F32, tag="out_t", name="out_t")
            out_t = out_tiles[s]
            g1, g2 = live.pop(g)

            ca = ci % A_CHUNKS
            if ca == 0:
                live["A"] = pps_a.tile([128, A_W], F32, tag="A", name="ps_a")
            ps_a = live["A"]
            for q in range(CHUNK_POS):
                q4, qg = q % 4, q // 4 + 4 * ca
                nc.tensor.matmul(
                    ps_a[32 * q4:32 * q4 + C, C * qg:C * qg + C],
                    g1[:, C * q:C * q + C],
                    g2[:, C * q:C * q + C],
                    tile_position=(0, 32 * q4),
                )

            if ca == A_CHUNKS - 1:
                np_used = 3 * 32 + C  # 118 partitions carry real rows
                ngrp = 4 * A_CHUNKS
                e_sb = psm.tile([128, A_W], F32, tag="e")
                nc.scalar.activation(
                    e_sb[0:np_used, :], ps_a[0:np_used, :],
                    mybir.ActivationFunctionType.Exp, scale=SCALE,
                )
                e_v = e_sb[0:np_used, :].rearrange("p (g e) -> p g e", e=C)
                s_sb = psm.tile([128, ngrp], F32, tag="s")
                nc.vector.reduce_sum(s_sb[0:np_used, :], e_v,
                                     axis=mybir.AxisListType.X)
                r_sb = psm.tile([128, ngrp], F32, tag="r")
                nc.vector.reciprocal(r_sb[0:np_used, :], s_sb[0:np_used, :])

                o_v = out_t[0:np_used,
                            (ci + 1 - A_CHUNKS) * 4 * C:(ci + 1) * 4 * C
                            ].rearrange("p (g e) -> p g e", e=C)
                r_v = r_sb[0:np_used, :].unsqueeze(2)
                e_b, r_b = bass.broadcast_tensor_aps(e_v, r_v)
                nc.gpsimd.tensor_tensor(
                    out=o_v, in0=e_b, in1=r_b, op=mybir.AluOpType.mult)

            if ci == SB_CHUNKS - 1:
                for q4 in range(4):
                    nc.sync.dma_start(
                        out=y_ap[q4, :, s, :],
                        in_=out_t[32 * q4:32 * q4 + C, :],
                    )

        TOTAL = N_SB * SB_CHUNKS
        for g in range(TOTAL + 1):
            if g < TOTAL:
                emit_stage1(g)
            if g >= 1:
                emit_stage2(g - 1)

    nc.compile()
    return nc


def shard_inputs(inputs):
    freq = np.asarray(inputs["freq_feat"], dtype=np.float32)
    time = np.asarray(inputs["time_out"], dtype=np.float32)
    W1 = np.asarray(inputs["W1"], dtype=np.float32)
    W2 = np.asarray(inputs["W2"], dtype=np.float32)
    W3 = np.asarray(inputs["W3"], dtype=np.float32)
    W4 = np.asarray(inputs["W4"], dtype=np.float32)
    b1 = np.asarray(inputs["b1"], dtype=np.float32)
    b2 = np.asarray(inputs["b2"], dtype=np.float32)
    b3 = np.asarray(inputs["b3"], dtype=np.float32)
    b4 = np.asarray(inputs["b4"], dtype=np.float32)

    bf16 = ml_dtypes.bfloat16
    Z = np.zeros((HID, HID), dtype=np.float32)
    # wf_fold blocks (each [128, 64]): [W1T;0], [0;W1T], [W2T;0], [0;W2T]
    wf = np.concatenate([
        np.concatenate([W1.T, Z], axis=0),
        np.concatenate([Z, W1.T], axis=0),
        np.concatenate([W2.T, Z], axis=0),
        np.concatenate([Z, W2.T], axis=0),
    ], axis=1).astype(bf16)                                     # [128, 256]
    wt = np.concatenate([W3.T, W4.T], axis=1).astype(bf16)      # [128, 128]
    bias1 = np.concatenate([b1, b3]).reshape(128, 1).astype(np.float32)
    bias2 = np.concatenate([b2, b4]).reshape(128, 1).astype(np.float32)

    in_maps = []
    for i in range(N_CORES):
        fs = freq[B_LOC * i:B_LOC * (i + 1)]   # [512, 22, 64, 9]
        ts = time[B_LOC * i:B_LOC * (i + 1)]   # [512, 22, 128, 9]
        # [B, C, D, T] -> [D, B, T, C] -> [D, B*T*C]
        xf2 = fs.transpose(2, 0, 3, 1).reshape(HID, NCOL)
        xt2 = ts.transpose(2, 0, 3, 1).reshape(2 * HID, NCOL).astype(bf16)
        # fold: chunk-half pairing -> [128, NCOL/2]
        xf3 = xf2.reshape(HID, N_CHUNK, 2, HALF)
        xfold = np.concatenate([xf3[:, :, 0, :], xf3[:, :, 1, :]],
                               axis=0).reshape(128, NCOL // 2).astype(bf16)
        in_maps.append({
            "xf": np.ascontiguousarray(xfold),
            "xt": np.ascontiguousarray(xt2),
            "wf": wf, "wt": wt, "bias1": bias1, "bias2": bias2,
        })
    return in_maps


def unshard_output(y_dev):
    """[4, 22, 18, 1408] device layout -> [512, 9, 22, 22]."""
    a = y_dev.reshape(4, C, N_SB, SB_CHUNKS, 4, C)   # [q4, cc, s, ch, qg, e]
    a = a.transpose(2, 3, 4, 0, 1, 5)                # [s, ch, qg, q4, cc, e]
    return np.ascontiguousarray(a).reshape(B_LOC, T, C, C)


_NC_CACHE = None


def _get_program():
    global _NC_CACHE
    if _NC_CACHE is None:
        _NC_CACHE = build_program()
    return _NC_CACHE


def run(inputs, trace=False):
    nc = _get_program()
    in_maps = shard_inputs(inputs)
    res = run_bass_kernel_spmd(nc, in_maps, core_ids=list(range(N_CORES)),
                               trace=trace)
    outs = [unshard_output(np.asarray(r["y"], dtype=np.float32))
            for r in res.results]
    full = np.concatenate(outs, axis=0)  # [4096, 9, 22, 22]
    return full, res


def kernel(**inputs) -> np.ndarray:
    full, _ = run(inputs, trace=False)
    return full
